# revision 15
# baseline (speedup 1.0000x reference)
"""
AM-Softmax + intra-class loss kernel for Trainium2, 8 NeuronCores.

Strategy (class-sharded distributed softmax, fp8 DoubleRow matmul):
  * Classes C=20000 sharded 2500/core (padded 2560). Host normalizes E and W
    rows to unit norm (the per-row scale that fp8 quantization needs anyway),
    scales by 16 and casts to fp8-e4m3. The AM scale (30) and the two 1/16
    factors ride the exp's constant scale, so no norm work runs on device.
  * Z tiles come from fp8 DoubleRow matmuls: one instruction contracts the
    full K=256 (two 128-deep k-tiles) per 512-wide PSUM bank -- 4x the f32r
    rate on the PE.
  * Per 128-row chunk (32 chunks): ACT exps cols [0:1536] with the fused
    per-partition accumulator (exact exp, logsumexp offset -30); DVE covers
    cols [1536:2500] with a Schraudolph exp built for bf16: i16 = round(
    a*z + b) is bf16's bit pattern of ~exp(z-30), then one
    scalar_tensor_tensor folds the two halves and row-sums in a single
    pass. The ~1.5e-3 systematic error of the piecewise-linear exp sits far
    under the tolerance; the label-logit correction uses exact f32 dots.
  * Label logits: host gathers normalized W[labels]; device does 4 fused
    dot-product instructions (scalar_tensor_tensor accumulate).
  * Intra-class term: group-sum trick on normalized eg rows (bf16
    sel-matmul), sum((1 - e_i.e_j)) = 28 - (||sum_g e||^2 - 8)/2 per group.
  * Host combine is O(B) float64.
"""

import numpy as np
import ml_dtypes

import concourse.bacc as bacc
import concourse.tile as tile
from concourse import mybir
from concourse.bass_utils import run_bass_kernel_spmd
from contextlib import ExitStack

B = 4096
D = 256
C = 20000
G = 512
NSAMP = 8
NCORES = 8
CREAL = C // NCORES          # 2500 real classes per core
CSH = 2560                   # padded classes per core
RCH = B // 128               # 32 row chunks
RPC = B // NCORES            # 512 rows per core (label-cos shard)
GPC = G // NCORES            # 64 groups per core
NA = 1536                    # ACT cols per chunk (exact exp)
NB = CREAL - NA              # 964 DVE cols per chunk (Schraudolph)

AM_MARGIN = 0.3
AM_SCALE = 30.0
INTRA_MARGIN = 0.5
LAMBDA_INTRA = 0.1
OFF = 30.0                   # logsumexp offset
QS = 16.0                    # fp8 quantization scale on each operand
ZSCALE = AM_SCALE / (QS * QS)   # psum -> s*cos

# bf16-space Schraudolph: i16 = round(z*A16 + B16) is bf16 bits of ~exp(z)
A16 = float(2**7 / np.log(2))
B16 = float(127 * 2**7 - 0.927)
SCH_MUL = A16 * ZSCALE
SCH_ADD = B16 - A16 * OFF

F32 = mybir.dt.float32
BF16 = mybir.dt.bfloat16
F8 = mybir.dt.float8e4
I16 = mybir.dt.int16
AF = mybir.ActivationFunctionType
ALU = mybir.AluOpType
AXL = mybir.AxisListType
PM = mybir.MatmulPerfMode


def build_program():
    nc = bacc.Bacc("TRN2", target_bir_lowering=False)

    etq_d = nc.dram_tensor("etq", [128, 2, B], F8, kind="ExternalInput")
    wtq_d = nc.dram_tensor("wtq", [128, 2, CSH], F8, kind="ExternalInput")
    ern_d = nc.dram_tensor("ern", [128, 4, D], F32, kind="ExternalInput")
    wln_d = nc.dram_tensor("wln", [128, 4, D], F32, kind="ExternalInput")
    egn_d = nc.dram_tensor("egn", [128, 4, D], BF16, kind="ExternalInput")
    sel_d = nc.dram_tensor("sel", [128, GPC], BF16, kind="ExternalInput")

    outa_d = nc.dram_tensor("out_a", [128, RCH], F32, kind="ExternalOutput")
    outb_d = nc.dram_tensor("out_b", [128, RCH], F32, kind="ExternalOutput")
    outlc_d = nc.dram_tensor("out_lc", [128, 4], F32, kind="ExternalOutput")
    outiv_d = nc.dram_tensor("out_iv", [GPC, 1], F32, kind="ExternalOutput")

    with tile.TileContext(nc) as tc, ExitStack() as ctx:
        big = ctx.enter_context(tc.tile_pool(name="big", bufs=1))
        scr = ctx.enter_context(tc.tile_pool(name="scr", bufs=3))
        psum = ctx.enter_context(tc.tile_pool(name="psum", bufs=2, space="PSUM"))
        psg = ctx.enter_context(tc.tile_pool(name="psg", bufs=1, space="PSUM"))

        ETQ = big.tile([128, 2, B], F8)
        WTQ = big.tile([128, 2, CSH], F8)
        ernsb = big.tile([128, 4, D], F32)
        wlnsb = big.tile([128, 4, D], F32)
        egnsb = big.tile([128, 4, D], BF16)
        selsb = big.tile([128, GPC], BF16)

        # DMAs: SP queue carries the matmul operands in need order; the
        # gpsimd (SWDGE) queue carries the small tail tensors in parallel.
        nc.sync.dma_start(out=WTQ[:, :, 0:NA], in_=wtq_d[:][:, :, 0:NA])
        nc.sync.dma_start(out=ETQ[:, :, 0:1024], in_=etq_d[:][:, :, 0:1024])
        nc.sync.dma_start(out=WTQ[:, :, NA:CSH], in_=wtq_d[:][:, :, NA:CSH])
        for q in range(1, 4):
            nc.sync.dma_start(out=ETQ[:, :, q * 1024:(q + 1) * 1024],
                              in_=etq_d[:][:, :, q * 1024:(q + 1) * 1024])
        nc.sync.dma_start(out=selsb, in_=sel_d[:])
        nc.sync.dma_start(out=egnsb, in_=egn_d[:])
        nc.sync.dma_start(out=ernsb, in_=ern_d[:])
        nc.sync.dma_start(out=wlnsb, in_=wln_d[:])

        negoff = big.tile([128, 1], F32)
        nc.vector.memset(negoff, -OFF)
        tsA = big.tile([128, RCH], F32)
        tsB = big.tile([128, RCH], F32)
        lcpack = big.tile([128, 4], F32)

        npairs = NSAMP * (NSAMP - 1) / 2.0
        ssq = big.tile([GPC, 1], F32)
        iv = big.tile([GPC, 1], F32)

        # ---------------- main loop ----------------
        for r in range(RCH):
            if r == 2:
                # intra group-sum matmuls ride the PE while it idles early
                sg = psg.tile([GPC, D], F32)
                for j in range(4):
                    nc.tensor.matmul(sg, lhsT=selsb, rhs=egnsb[:, j],
                                     start=(j == 0), stop=(j == 3))
            if r == 4:
                # intra tail math rides early DVE gaps; frees the sg bank
                sgsb = scr.tile([GPC, D], F32, tag="sgsb")
                nc.vector.tensor_copy(out=sgsb, in_=sg)
                junk2 = scr.tile([GPC, D], F32, tag="sgj")
                nc.vector.scalar_tensor_tensor(
                    out=junk2, in0=sgsb, scalar=1.0, in1=sgsb,
                    op0=ALU.mult, op1=ALU.mult, accum_out=ssq)
                nc.vector.tensor_scalar(out=iv, in0=ssq,
                                        scalar1=-1.0 / (2.0 * npairs),
                                        scalar2=(1.0 - INTRA_MARGIN) + NSAMP / (2.0 * npairs),
                                        op0=ALU.mult, op1=ALU.add)
                nc.vector.tensor_scalar_max(iv, iv, 0.0)
                nc.sync.dma_start(out=outiv_d[:], in_=iv)
            if r in (8, 14, 20, 26):
                # one label-cos dot per slot, filling DVE pipeline gaps
                j = (r - 8) // 6
                junk = scr.tile([128, D], F32, tag="lcj")
                nc.vector.scalar_tensor_tensor(
                    out=junk, in0=ernsb[:, j], scalar=1.0, in1=wlnsb[:, j],
                    op0=ALU.mult, op1=ALU.mult, accum_out=lcpack[:, j:j + 1])
            lhs = ETQ[:, :, r * 128:(r + 1) * 128]
            # B tile first: its PSUM slot recycles off the DVE chain alone,
            # so the next chunk's B matmuls never wait on ACT.
            ptB = psum.tile([128, 1024], F32, tag="mm")
            for tb in range(2):
                nc.tensor.matmul(ptB[:, tb * 512:(tb + 1) * 512], lhsT=lhs,
                                 rhs=WTQ[:, :, NA + tb * 512:NA + (tb + 1) * 512],
                                 start=True, stop=True, perf_mode=PM.DoubleRow)
            # A tile: cols 0:1536, exact exp on ACT with fused row-accum
            ptA = psum.tile([128, NA], F32, tag="mm")
            for tb in range(3):
                nc.tensor.matmul(ptA[:, tb * 512:(tb + 1) * 512], lhsT=lhs,
                                 rhs=WTQ[:, :, tb * 512:(tb + 1) * 512],
                                 start=True, stop=True, perf_mode=PM.DoubleRow)
            # Schraudolph transform of the 964 real cols; fused halves-add +
            # row-sum via scalar_tensor_tensor.
            sch = scr.tile([128, 1024], I16, tag="sch")
            nc.vector.tensor_scalar(out=sch[:, 0:NB], in0=ptB[:, 0:NB],
                                    scalar1=SCH_MUL, scalar2=SCH_ADD,
                                    op0=ALU.mult, op1=ALU.add)
            h = NB // 2  # 482
            stsc = scr.tile([128, h], BF16, tag="stsc")
            nc.vector.scalar_tensor_tensor(
                out=stsc, in0=sch.bitcast(BF16)[:, 0:h], scalar=1.0,
                in1=sch.bitcast(BF16)[:, h:NB],
                op0=ALU.mult, op1=ALU.add, accum_out=tsB[:, r:r + 1])
            sA = scr.tile([128, NA], F32, tag="expA")
            nc.scalar.activation(out=sA, in_=ptA, func=AF.Exp,
                                 scale=ZSCALE, bias=negoff[:, 0:1],
                                 accum_out=tsA[:, r:r + 1])
            if r == RCH - 8:
                # drain the filled portion of the sums early so the final
                # output DMAs only carry the last columns
                nc.sync.dma_start(out=outa_d[:][:, 0:RCH - 8], in_=tsA[:, 0:RCH - 8])
                nc.sync.dma_start(out=outb_d[:][:, 0:RCH - 8], in_=tsB[:, 0:RCH - 8])

        nc.sync.dma_start(out=outa_d[:][:, RCH - 8:RCH], in_=tsA[:, RCH - 8:RCH])
        nc.sync.dma_start(out=outb_d[:][:, RCH - 8:RCH], in_=tsB[:, RCH - 8:RCH])
        nc.sync.dma_start(out=outlc_d[:], in_=lcpack)

    nc.finalize()
    return nc


def kernel(embeddings, labels, weight):
    e = np.ascontiguousarray(embeddings, dtype=np.float32)
    lab = np.asarray(labels).astype(np.int64)
    w = np.ascontiguousarray(weight, dtype=np.float32)
    assert e.shape == (B, D) and w.shape == (C, D) and lab.shape == (B,)

    En = (e / np.linalg.norm(e, axis=1, keepdims=True)).astype(np.float32)
    Wn = (w / np.linalg.norm(w, axis=1, keepdims=True)).astype(np.float32)
    Eq = (QS * En).astype(ml_dtypes.float8_e4m3fn)
    etq = np.ascontiguousarray(
        Eq.T.reshape(2, 128, B).transpose(1, 0, 2))          # [128, 2, B]

    members = np.argsort(lab, kind="stable").reshape(G, NSAMP)
    assert np.all(lab[members[:, 0]] == np.arange(G))
    sel = np.tile(np.eye(GPC, dtype=np.float32), (2, 1)).astype(ml_dtypes.bfloat16)

    in_maps = []
    for k in range(NCORES):
        wsh = np.zeros((CSH, D), np.float32)
        wsh[:CREAL] = Wn[k * CREAL:(k + 1) * CREAL]
        Wq = (QS * wsh).astype(ml_dtypes.float8_e4m3fn)
        wtq = np.ascontiguousarray(Wq.T.reshape(2, 128, CSH).transpose(1, 0, 2))
        rows = slice(k * RPC, (k + 1) * RPC)
        ern = np.ascontiguousarray(
            En[rows].reshape(4, 128, D).transpose(1, 0, 2))
        wln = np.ascontiguousarray(
            Wn[lab[rows]].reshape(4, 128, D).transpose(1, 0, 2))
        gm = members[k * GPC:(k + 1) * GPC]
        eg_idx = gm.T.reshape(-1)
        egn = np.ascontiguousarray(
            En[eg_idx].reshape(4, 128, D).transpose(1, 0, 2)
        ).astype(ml_dtypes.bfloat16)
        in_maps.append({
            "etq": etq, "wtq": wtq, "ern": ern, "wln": wln,
            "egn": egn, "sel": sel,
        })

    nc = build_program()
    res = run_bass_kernel_spmd(nc, in_maps, core_ids=list(range(NCORES)))
    global _last_results
    _last_results = res

    # ---------------- host combine (O(B), float64) -----------------------
    S = np.zeros(B, np.float64)
    for k in range(NCORES):
        rk = res.results[k]
        S += (rk["out_a"].astype(np.float64) +
              rk["out_b"].astype(np.float64)).T.reshape(B)
    cl = np.concatenate(
        [res.results[k]["out_lc"].astype(np.float64).T.reshape(RPC)
         for k in range(NCORES)])

    s, m = float(AM_SCALE), float(AM_MARGIN)
    S_adj = S - np.exp(s * cl - OFF) + np.exp(s * (cl - m) - OFF)
    am_i = (np.log(S_adj) + OFF) - s * (cl - m)
    am = am_i.mean()

    ivals = np.concatenate(
        [res.results[k]["out_iv"][:, 0] for k in range(NCORES)]
    ).astype(np.float64)
    intra = ivals.sum() / G
    total = am + LAMBDA_INTRA * intra
    return (np.float32(total), np.float32(am), np.float32(intra))
